# revision 22
# baseline (speedup 1.0000x reference)
"""BitLinear inference kernel for Trainium2, sharded over 8 NeuronCores.

Computes, per the reference:
    w_q = sign(w - mean(w));  w_scale = mean(|w|)
    b_q = sign(b - mean(b));  b_scale = mean(|b|)
    xn  = x / max(||x||_2, 1e-12) * D**-0.5            (per token)
    sc  = 127 / max(max|xn|, 1e-5)                     (per token)
    x_q = clip(round(xn * sc), -128, 127)
    y   = (x_q @ w_q.T + b_q) / (w_scale * sc * b_scale)

Sharding: x/y split into 8 contiguous row blocks of 4096 tokens (data
parallel over B*S); w, b replicated.  All per-token math is on-core.

v2 design (PE-roofline oriented): the bf16 matmul itself is the hard
floor (~3.4us per 128-token tile); everything else is moved off the PE:
  - x is uploaded in bf16 and y stored in bf16 (host casts); w uploaded
    PRE-TRANSPOSED [D, O] in bf16.  Halves all DMA traffic and kills
    the PE w-transposes; numerics stay ~100x under the 2e-2 gate.
  - x_q (bf16, integers <=127 exact) is transposed by the xbar DMA
    transpose engine straight into SBUF, not by PE matmul-transposes.
  - the bias row b_q is pre-broadcast to [128, O] once, and per tile
    seeded into PSUM by the ACT engine; all matmuls run start=False.
  - round-half-to-even via the +-1.5*2^23 magic constant on DVE; the
    l2 norm cancels in x_q so quant only needs 127/amax.
  - dequant scale needs 1/||x||: DVE reciprocal + ACT sqrt seed + two
    Newton rsqrt refinements (exactly the v1 recipe).
"""

import os
import sys

import numpy as np

for _p in ("/opt/trn_rl_repo", "/root/.axon_site/_ro/trn_rl_repo"):
    if os.path.isdir(_p) and _p not in sys.path:
        sys.path.insert(0, _p)

import ml_dtypes

import concourse.bacc as bacc
import concourse.tile as tile
from concourse import mybir
from concourse.bass_utils import run_bass_kernel_spmd

F32 = mybir.dt.float32
BF16 = mybir.dt.bfloat16
FP16 = mybir.dt.float16
ALU = mybir.AluOpType
ACTF = mybir.ActivationFunctionType
BF16NP = ml_dtypes.bfloat16

N_CORES = 8
B, S, D, O = 4, 8192, 1024, 1024
TOKENS = B * S
TOK_PER_CORE = TOKENS // N_CORES          # 4096
P = 128                                   # partitions / token tile
NTILES = TOK_PER_CORE // P                # 32
DCH = D // P                              # 8 contraction chunks

MAGIC = 1.5 * 2.0**23                     # f32 round-to-nearest-even constant
OFFH = 1536.0                             # fp16 magic: x_q+1536 in ulp=1 window
DIM_SCALE = float(D) ** -0.5              # 2**-5, exact power of two
EPS_NORM_SQ = 1e-24                       # (1e-12)**2, matches l2 clamp
EPS_SCALE = 1e-5

# ---- tunables (overridable per-build via cfg) ----
CFG_DEFAULTS = dict(
    transp="pe",      # "xbar" (DMA transpose engine) | "pe"
    tring="act",      # xbar issue ring: "sp" | "act"
    bias="act",       # "act" (ACT seeds PSUM) | "pe" (rank-1 matmul)
    load="sp",        # x load ring: "sp" | "pool"
    store="sp",       # y store ring: "sp" | "act"
    group=4,          # token tiles per stats group
    psbufs=2,         # PSUM y-tile buffers
    qmode="dve1",     # "dve1" (1-op DVE) | "actcast" (DVE t1 + ACT cast)
                      # | "dve2" (DVE t1 + DVE cast)
    sqeng="dve",      # sumsq engine: "dve" (fused ttr) | "act" (Square)
    cp="dve",         # xqT PSUM->SBUF copy: "dve" | "act" | "split"
    skip=(),          # stages to skip (timing ablation only; wrong results)
)
_CFG = dict(CFG_DEFAULTS)


def build_module(repeat: int = 1, cfg: dict | None = None):
    global _CFG
    saved = _CFG
    _CFG = dict(CFG_DEFAULTS)
    if cfg:
        _CFG.update(cfg)
    try:
        return _build_module_inner(repeat)
    finally:
        _CFG = saved


def _build_module_inner(repeat: int):
    C = _CFG
    GROUP = C["group"]
    NGROUPS = NTILES // GROUP
    SKIP = set(C["skip"])

    nc = bacc.Bacc("TRN2", target_bir_lowering=False, debug=False)

    x_d = nc.dram_tensor("x", [TOK_PER_CORE, D], BF16, kind="ExternalInput")
    wt_d = nc.dram_tensor("wt", [D, O], F32, kind="ExternalInput")
    b_d = nc.dram_tensor("b", [O], F32, kind="ExternalInput")
    y_d = nc.dram_tensor("y", [TOK_PER_CORE, O], BF16, kind="ExternalOutput")

    x_r = x_d.ap().rearrange("(a p) d -> p a d", p=P)    # [128, 32, 1024]
    y_r = y_d.ap().rearrange("(a p) d -> p a d", p=P)
    wt_r = wt_d.ap().rearrange("(c p) o -> p c o", p=P)  # [128, 8, 1024]
    b_r = b_d.ap().rearrange("(o d) -> o d", o=1)        # [1, 1024]

    with tile.TileContext(nc) as tc:
        import contextlib

        with contextlib.ExitStack() as ctx:
            consts = ctx.enter_context(tc.tile_pool(name="consts", bufs=1))
            wpool = ctx.enter_context(tc.tile_pool(name="wpool", bufs=1))
            wtpool = ctx.enter_context(tc.tile_pool(name="wtpool", bufs=1))
            xpool = ctx.enter_context(tc.tile_pool(name="xpool", bufs=3))
            scr = ctx.enter_context(tc.tile_pool(name="scr", bufs=2))
            tpool = ctx.enter_context(tc.tile_pool(name="tpool", bufs=3))
            qpool = ctx.enter_context(tc.tile_pool(name="qpool", bufs=4))
            xtpool = ctx.enter_context(tc.tile_pool(name="xtpool", bufs=6))
            ypool = ctx.enter_context(tc.tile_pool(name="ypool", bufs=3))
            stats = ctx.enter_context(tc.tile_pool(name="stats", bufs=3))
            pspool = ctx.enter_context(
                tc.tile_pool(name="pspool", bufs=C["psbufs"], space="PSUM")
            )
            wps = ctx.enter_context(tc.tile_pool(name="wps", bufs=1, space="PSUM"))
            xps = None
            if C["transp"] == "pe":
                xps = ctx.enter_context(
                    tc.tile_pool(name="xps", bufs=2, space="PSUM")
                )

            # ---------------- constants ----------------
            ones_row = consts.tile([1, P], BF16)
            nc.vector.memset(ones_row, 1.0)
            ones128 = consts.tile([P, P], F32)
            nc.vector.memset(ones128, 1.0)
            if C["transp"] == "pe":
                from concourse.masks import make_identity

                identity_bf = consts.tile([P, P], FP16)
                make_identity(nc, identity_bf)

            # ---------------- weight/bias prep ----------------
            def emit_prep():
                # bias first; must not queue behind the 2MB w load
                b_sb = consts.tile([1, O], F32)
                nc.sync.dma_start(out=b_sb, in_=b_r)

                w_sb = wpool.tile([P, DCH, O], F32)
                for half in range(4):
                    nc.sync.dma_start(
                        out=w_sb[:, half * 2 : half * 2 + 2, :],
                        in_=wt_r[:, half * 2 : half * 2 + 2, :],
                    )

                # per-partition-row sums; split across ACT and DVE
                wsum = consts.tile([P, DCH], F32)
                wabs = consts.tile([P, DCH], F32)
                for r in range(DCH):
                    if r % 2 == 0:
                        dump = scr.tile([P, O], BF16, tag="sq")
                        nc.scalar.activation(
                            out=dump, in_=w_sb[:, r, :], func=ACTF.Copy,
                            accum_out=wsum[:, r : r + 1],
                        )
                    else:
                        nc.vector.tensor_reduce(
                            out=wsum[:, r : r + 1], in_=w_sb[:, r, :],
                            axis=mybir.AxisListType.X, op=ALU.add,
                        )
                for r in range(DCH):
                    nc.vector.tensor_reduce(
                        out=wabs[:, r : r + 1], in_=w_sb[:, r, :],
                        axis=mybir.AxisListType.X, op=ALU.add,
                        apply_absolute_value=True,
                    )
                w12 = consts.tile([P, 2], F32)
                nc.vector.tensor_reduce(
                    out=w12[:, 0:1], in_=wsum, axis=mybir.AxisListType.X,
                    op=ALU.add,
                )
                nc.vector.tensor_reduce(
                    out=w12[:, 1:2], in_=wabs, axis=mybir.AxisListType.X,
                    op=ALU.add,
                )
                # cross-partition reduce + broadcast in one f32 ones-matmul
                # (col 2 is reused later for the b_scale broadcast)
                statps = wps.tile([P, 4], F32, tag="stat", name="statps")
                nc.tensor.matmul(
                    statps[:, 0:2], lhsT=ones128, rhs=w12,
                    start=True, stop=True,
                )
                neg_mean_w = consts.tile([P, 1], F32)
                w_scale = consts.tile([P, 1], F32)
                nc.vector.tensor_scalar(
                    out=neg_mean_w, in0=statps[:, 0:1],
                    scalar1=-1.0 / float(O * D), scalar2=None, op0=ALU.mult,
                )
                nc.vector.tensor_scalar(
                    out=w_scale, in0=statps[:, 1:2],
                    scalar1=1.0 / float(O * D), scalar2=None, op0=ALU.mult,
                )

                # wqT[:, c, :] = Sign(wT_c - mean) straight from SBUF
                wqT = wtpool.tile([P, DCH, O], FP16)
                for c in range(DCH):
                    nc.scalar.activation(
                        out=wqT[:, c, :], in_=w_sb[:, c, :], func=ACTF.Sign,
                        bias=neg_mean_w, scale=1.0,
                    )

                # bias stats
                bsum = consts.tile([1, 1], F32)
                babs = consts.tile([1, 1], F32)
                nc.vector.tensor_reduce(
                    out=bsum, in_=b_sb, axis=mybir.AxisListType.X, op=ALU.add
                )
                nc.vector.tensor_reduce(
                    out=babs, in_=b_sb, axis=mybir.AxisListType.X, op=ALU.add,
                    apply_absolute_value=True,
                )
                neg_mean_b = consts.tile([1, 1], F32)
                b_scale1 = consts.tile([1, 1], F32)
                nc.vector.tensor_scalar(
                    out=neg_mean_b, in0=bsum, scalar1=-1.0 / float(O),
                    scalar2=None, op0=ALU.mult,
                )
                nc.vector.tensor_scalar(
                    out=b_scale1, in0=babs, scalar1=1.0 / float(O),
                    scalar2=None, op0=ALU.mult,
                )
                bq = consts.tile([1, O], BF16)
                nc.scalar.activation(
                    out=bq, in_=b_sb, func=ACTF.Sign, bias=neg_mean_b, scale=1.0
                )

                # PSUM seed row: b_q - 1536 * rowsum(w_q) per output column.
                # The quant step emits x_q + 1536 (fp16 magic), so the matmul
                # adds 1536*rowsum(wq) per column; the seed cancels it
                # exactly (all-integer f32).
                ones_colp = consts.tile([P, 1], FP16)
                nc.vector.memset(ones_colp, 1.0)
                rsh = wps.tile([1, 512], F32, tag="rs", name="rsps")
                seedrow = consts.tile([1, O], F32)
                for h in range(2):
                    for c in range(DCH):
                        nc.tensor.matmul(
                            rsh,
                            lhsT=ones_colp,
                            rhs=wqT[:, c, h * 512 : (h + 1) * 512],
                            start=(c == 0), stop=(c == DCH - 1),
                        )
                    nc.vector.tensor_scalar(
                        out=seedrow[:, h * 512 : (h + 1) * 512], in0=rsh,
                        scalar1=-OFFH, scalar2=None, op0=ALU.mult,
                    )
                bqf = consts.tile([1, O], F32)
                nc.vector.tensor_copy(out=bqf, in_=bq)
                seedrow2 = consts.tile([1, O], F32)
                nc.vector.tensor_tensor(
                    out=seedrow2, in0=seedrow, in1=bqf, op=ALU.add
                )
                # broadcast to all 128 partitions (f32 for exactness),
                # borrowing a main-loop PSUM buffer
                ones_col_f = consts.tile([1, P], F32)
                nc.vector.memset(ones_col_f, 1.0)
                bps = pspool.tile([P, O], F32, tag="ps", name="bps")
                for h in range(2):
                    nc.tensor.matmul(
                        bps[:, h * 512 : (h + 1) * 512],
                        lhsT=ones_col_f,
                        rhs=seedrow2[:, h * 512 : (h + 1) * 512],
                        start=True, stop=True,
                    )
                bqb = consts.tile([P, O], F32)
                nc.scalar.copy(out=bqb, in_=bps)

                # invc = 1 / (127 * w_scale * b_scale), broadcast to [128,1]
                nc.tensor.matmul(
                    statps[:, 2:3], lhsT=ones_col_f, rhs=b_scale1,
                    start=True, stop=True,
                )
                wb = consts.tile([P, 1], F32)
                nc.vector.tensor_tensor(
                    out=wb, in0=w_scale, in1=statps[:, 2:3], op=ALU.mult
                )
                wb127 = consts.tile([P, 1], F32)
                nc.vector.tensor_scalar(
                    out=wb127, in0=wb, scalar1=127.0, scalar2=None, op0=ALU.mult
                )
                invc = consts.tile([P, 1], F32)
                nc.vector.reciprocal(out=invc, in_=wb127)
                return wqT, bq, bqb, invc

            # ---------------- main loop ----------------
            # Software-pipelined: tile t's quant/transpose/PSUM-seed (front)
            # is emitted one tile AHEAD of tile t's matmuls+epilogue (back),
            # so the PE never waits for the xqT copy-back, and the ACT
            # stream orders seed(t+1) BEFORE epi(t).
            def emit_load(g):
                ldeng = nc.sync if C["load"] == "sp" else nc.gpsimd
                xg = xpool.tile([P, GROUP, D], BF16, tag="xg", name=f"xg_{g}")
                ldeng.dma_start(
                    out=xg, in_=x_r[:, g * GROUP : (g + 1) * GROUP, :]
                )
                sumsq = stats.tile([P, GROUP], F32, tag="sumsq", name=f"sumsq_{g}")
                amax = stats.tile([P, GROUP], F32, tag="amax", name=f"amax_{g}")
                return [xg, sumsq, amax, None, None]

            def emit_sqamax(st, j):
                xg, sumsq, amax = st[0], st[1], st[2]
                if "stats" in SKIP:
                    return
                sq = scr.tile([P, D], BF16, tag="sq")
                nc.scalar.activation(
                    out=sq, in_=xg[:, j, :], func=ACTF.Square,
                    accum_out=sumsq[:, j : j + 1],
                )
                nc.vector.tensor_reduce(
                    out=amax[:, j : j + 1], in_=xg[:, j, :],
                    axis=mybir.AxisListType.X, op=ALU.max,
                    apply_absolute_value=True,
                )

            def emit_chain(st, prep):
                wqT, bq, bqb, invc = prep
                xg, sumsq, amax = st[0], st[1], st[2]
                # per-token scalar chain on [128, GROUP]
                m = stats.tile([P, GROUP], F32, tag="m")
                gsc = stats.tile([P, GROUP], F32, tag="gsc")
                if "stats" in SKIP:
                    nc.vector.memset(m, 1.0)
                    nc.vector.memset(gsc, 1.0)
                    st[3], st[4] = m, gsc
                    return
                ssq = stats.tile([P, GROUP], F32)
                nc.vector.tensor_scalar(
                    out=ssq, in0=sumsq, scalar1=EPS_NORM_SQ, scalar2=None,
                    op0=ALU.max,
                )
                u = stats.tile([P, GROUP], F32)
                nc.vector.reciprocal(out=u, in_=ssq)
                v = stats.tile([P, GROUP], F32)
                nc.scalar.activation(out=v, in_=u, func=ACTF.Sqrt)
                for _ in range(2):  # Newton rsqrt refinement
                    rr = stats.tile([P, GROUP], F32, tag="rr")
                    nc.vector.tensor_tensor(out=rr, in0=v, in1=v, op=ALU.mult)
                    qq = stats.tile([P, GROUP], F32, tag="qq")
                    nc.vector.tensor_tensor(out=qq, in0=rr, in1=ssq, op=ALU.mult)
                    ww = stats.tile([P, GROUP], F32, tag="ww")
                    nc.vector.tensor_scalar(
                        out=ww, in0=qq, scalar1=-0.5, scalar2=1.5,
                        op0=ALU.mult, op1=ALU.add,
                    )
                    v2 = stats.tile([P, GROUP], F32, tag="vv")
                    nc.vector.tensor_tensor(out=v2, in0=v, in1=ww, op=ALU.mult)
                    v = v2

                am = stats.tile([P, GROUP], F32)
                nc.vector.tensor_scalar(
                    out=am, in0=amax, scalar1=1e-30, scalar2=None, op0=ALU.max,
                )
                im = stats.tile([P, GROUP], F32)
                nc.vector.reciprocal(out=im, in_=am)
                nc.vector.tensor_scalar(
                    out=m, in0=im, scalar1=127.0, scalar2=None, op0=ALU.mult,
                )
                ax1 = stats.tile([P, GROUP], F32)
                nc.vector.tensor_tensor(out=ax1, in0=amax, in1=v, op=ALU.mult)
                axnc = stats.tile([P, GROUP], F32)
                nc.vector.tensor_scalar(
                    out=axnc, in0=ax1, scalar1=DIM_SCALE, scalar2=EPS_SCALE,
                    op0=ALU.mult, op1=ALU.max,
                )
                nc.vector.tensor_scalar(
                    out=gsc, in0=axnc, scalar1=invc, scalar2=None, op0=ALU.mult,
                )
                st[3], st[4] = m, gsc

            def emit_front(t, st, prep):
                wqT, bq, bqb, invc = prep
                teng = nc.scalar if C["tring"] == "act" else nc.sync
                xg, m, gsc = st[0], st[3], st[4]
                j = t % GROUP

                # quantize: xq_pre = x_q + 1536, fp16 (ulp=1 in [1024,2048),
                # so the fp16 output cast IS the round-half-even)
                xq = qpool.tile([P, D], FP16)
                if "quant" in SKIP:
                    nc.vector.memset(xq, OFFH)
                elif C["qmode"] == "dve1":
                    nc.vector.tensor_scalar(
                        out=xq, in0=xg[:, j, :], scalar1=m[:, j : j + 1],
                        scalar2=OFFH, op0=ALU.mult, op1=ALU.add,
                    )
                else:
                    # f32 magic round on DVE, then exact fp16 cast (+1536)
                    t1 = tpool.tile([P, D], F32)
                    nc.vector.tensor_scalar(
                        out=t1, in0=xg[:, j, :], scalar1=m[:, j : j + 1],
                        scalar2=MAGIC, op0=ALU.mult, op1=ALU.add,
                    )
                    if C["qmode"] == "actcast":
                        nc.scalar.activation(
                            out=xq, in_=t1, func=ACTF.Copy,
                            bias=OFFH - MAGIC, scale=1.0,
                        )
                    else:
                        nc.vector.tensor_scalar(
                            out=xq, in0=t1, scalar1=MAGIC - OFFH,
                            scalar2=None, op0=ALU.subtract,
                        )

                # transpose -> xqT[p, c, t] = xq[t, c*128+p]
                xqT = xtpool.tile([P, DCH, P], FP16)
                if "transpose" in SKIP:
                    nc.vector.memset(xqT, OFFH)
                elif C["transp"] == "xbar":
                    teng.dma_start_transpose(xqT, xq)
                else:
                    ptx = xps.tile([P, D], FP16, tag="xtp")
                    for c in range(DCH):
                        nc.tensor.transpose(
                            ptx[:, c * P : (c + 1) * P],
                            xq[:, c * P : (c + 1) * P],
                            identity_bf,
                        )
                    xqT_flat = xqT.rearrange("p c t -> p (c t)")
                    if C["cp"] == "dve":
                        nc.vector.tensor_copy(out=xqT_flat, in_=ptx)
                    elif C["cp"] == "act":
                        nc.scalar.copy(out=xqT_flat, in_=ptx)
                    else:
                        nc.vector.tensor_copy(
                            out=xqT_flat[:, 0:512], in_=ptx[:, 0:512]
                        )
                        nc.scalar.copy(
                            out=xqT_flat[:, 512:1024], in_=ptx[:, 512:1024]
                        )

                # seed PSUM with bias - 384*rowsum(wq) (ACT)
                ps = pspool.tile([P, O], F32, tag="ps")
                if "mm" not in SKIP:
                    nc.scalar.copy(out=ps, in_=bqb)
                return xqT, ps, gsc, j

            def emit_back(t, rec, prep, ycarry):
                wqT, bq, bqb, invc = prep
                steng = nc.sync if C["store"] == "sp" else nc.scalar
                xqT, ps, gsc, j = rec
                pss = [ps[:, 0:512], ps[:, 512:1024]]

                if "mm" not in SKIP:
                    for c in range(DCH):
                        for h in range(2):
                            nc.tensor.matmul(
                                pss[h],
                                lhsT=xqT[:, c, :],
                                rhs=wqT[:, c, h * 512 : (h + 1) * 512],
                                start=False,
                                stop=(c == DCH - 1),
                                skip_group_check=(c == 0),
                            )

                # dequant + store (bf16 out), batched x2
                if t % 2 == 0:
                    ycarry["yt2"] = ypool.tile(
                        [P, 2, O], BF16, tag="yt", name=f"yt2_{t}"
                    )
                yt2 = ycarry["yt2"]
                if "epi" not in SKIP and "mm" not in SKIP:
                    nc.scalar.activation(
                        out=yt2[:, t % 2, :], in_=ps, func=ACTF.Copy,
                        bias=0.0, scale=gsc[:, j : j + 1],
                    )
                else:
                    nc.vector.memset(yt2[:, t % 2, :], 0.0)
                if t % 2 == 1:
                    steng.dma_start(
                        out=y_r[:, t - 1 : t + 1, :], in_=yt2
                    )

            def main_loop(prep):
                ycarry = {}
                pending = None
                states = {}
                # prologue: group 0 stats upfront
                states[0] = emit_load(0)
                for j in range(GROUP):
                    emit_sqamax(states[0], j)
                emit_chain(states[0], prep)
                for t in range(NTILES):
                    g, k = divmod(t, GROUP)
                    if k == 0 and g + 1 < NGROUPS:
                        states[g + 1] = emit_load(g + 1)
                    rec = emit_front(t, states[g], prep)
                    # spread next group's stats one op per tile slot
                    if g + 1 < NGROUPS:
                        emit_sqamax(states[g + 1], k)
                        if k == GROUP - 1:
                            emit_chain(states[g + 1], prep)
                    if pending is not None:
                        emit_back(t - 1, pending, prep, ycarry)
                    pending = rec
                    states.pop(g - 1, None)
                emit_back(NTILES - 1, pending, prep, ycarry)

            if repeat == 1:
                prep = emit_prep()
                main_loop(prep)
            else:
                prep = emit_prep()
                with tc.For_i(0, repeat, 1):
                    main_loop(prep)

    nc.compile()
    return nc


_NC_CACHE = None


def _get_module():
    global _NC_CACHE
    if _NC_CACHE is None:
        _NC_CACHE = build_module()
    return _NC_CACHE


def make_in_map(x_core: np.ndarray, w: np.ndarray, b: np.ndarray) -> dict:
    """Per-core input map: x block in bf16, w transposed+bf16, b f32."""
    return {
        "x": np.ascontiguousarray(x_core, dtype=BF16NP),
        "wt": np.ascontiguousarray(np.asarray(w, dtype=np.float32).T),
        "b": np.ascontiguousarray(b, dtype=np.float32),
    }


def kernel(x: np.ndarray, w: np.ndarray, b: np.ndarray) -> np.ndarray:
    assert x.shape == (B, S, D) and w.shape == (O, D) and b.shape == (O,)
    nc = _get_module()

    xf = np.asarray(x, dtype=np.float32).reshape(TOKENS, D).astype(BF16NP)
    wt = np.ascontiguousarray(np.asarray(w, dtype=np.float32).T)
    bf = np.ascontiguousarray(b, dtype=np.float32)

    in_maps = [
        {
            "x": np.ascontiguousarray(
                xf[i * TOK_PER_CORE : (i + 1) * TOK_PER_CORE]
            ),
            "wt": wt,
            "b": bf,
        }
        for i in range(N_CORES)
    ]
    res = run_bass_kernel_spmd(nc, in_maps, core_ids=list(range(N_CORES)))
    out = np.concatenate([res.results[i]["y"] for i in range(N_CORES)], axis=0)
    return out.reshape(B, S, O).astype(np.float32)


# revision 23
# speedup vs baseline: 1.1698x; 1.1698x over previous
"""BitLinear inference kernel for Trainium2, sharded over 8 NeuronCores.

Computes, per the reference:
    w_q = sign(w - mean(w));  w_scale = mean(|w|)
    b_q = sign(b - mean(b));  b_scale = mean(|b|)
    xn  = x / max(||x||_2, 1e-12) * D**-0.5            (per token)
    sc  = 127 / max(max|xn|, 1e-5)                     (per token)
    x_q = clip(round(xn * sc), -128, 127)
    y   = (x_q @ w_q.T + b_q) / (w_scale * sc * b_scale)

Sharding: x/y split into 8 contiguous row blocks of 4096 tokens (data
parallel over B*S); w, b replicated.  All per-token math is on-core.

v2 design (PE-roofline oriented): the bf16 matmul itself is the hard
floor (~3.4us per 128-token tile); everything else is moved off the PE:
  - x is uploaded in bf16 and y stored in bf16 (host casts); w uploaded
    PRE-TRANSPOSED [D, O] in bf16.  Halves all DMA traffic and kills
    the PE w-transposes; numerics stay ~100x under the 2e-2 gate.
  - x_q (bf16, integers <=127 exact) is transposed by the xbar DMA
    transpose engine straight into SBUF, not by PE matmul-transposes.
  - the bias row b_q is pre-broadcast to [128, O] once, and per tile
    seeded into PSUM by the ACT engine; all matmuls run start=False.
  - round-half-to-even via the +-1.5*2^23 magic constant on DVE; the
    l2 norm cancels in x_q so quant only needs 127/amax.
  - dequant scale needs 1/||x||: DVE reciprocal + ACT sqrt seed + two
    Newton rsqrt refinements (exactly the v1 recipe).
"""

import os
import sys

import numpy as np

for _p in ("/opt/trn_rl_repo", "/root/.axon_site/_ro/trn_rl_repo"):
    if os.path.isdir(_p) and _p not in sys.path:
        sys.path.insert(0, _p)

import ml_dtypes

import concourse.bacc as bacc
import concourse.tile as tile
from concourse import mybir
from concourse.bass_utils import run_bass_kernel_spmd

F32 = mybir.dt.float32
BF16 = mybir.dt.bfloat16
FP16 = mybir.dt.float16
ALU = mybir.AluOpType
ACTF = mybir.ActivationFunctionType
BF16NP = ml_dtypes.bfloat16

N_CORES = 8
B, S, D, O = 4, 8192, 1024, 1024
TOKENS = B * S
TOK_PER_CORE = TOKENS // N_CORES          # 4096
P = 128                                   # partitions / token tile
NTILES = TOK_PER_CORE // P                # 32
DCH = D // P                              # 8 contraction chunks

MAGIC = 1.5 * 2.0**23                     # f32 round-to-nearest-even constant
OFFH = 1536.0                             # fp16 magic: x_q+1536 in ulp=1 window
DIM_SCALE = float(D) ** -0.5              # 2**-5, exact power of two
EPS_NORM_SQ = 1e-24                       # (1e-12)**2, matches l2 clamp
EPS_SCALE = 1e-5

# ---- tunables (overridable per-build via cfg) ----
CFG_DEFAULTS = dict(
    transp="pe",      # "xbar" (DMA transpose engine) | "pe"
    tring="act",      # xbar issue ring: "sp" | "act"
    bias="act",       # "act" (ACT seeds PSUM) | "pe" (rank-1 matmul)
    load="sp",        # x load ring: "sp" | "pool"
    store="sp",       # y store ring: "sp" | "act"
    group=4,          # token tiles per stats group
    psbufs=2,         # PSUM y-tile buffers
    qmode="dve1",     # "dve1" (1-op DVE) | "actcast" (DVE t1 + ACT cast)
                      # | "dve2" (DVE t1 + DVE cast)
    sqeng="dve",      # sumsq engine: "dve" (fused ttr) | "act" (Square)
    cp="dve",         # xqT PSUM->SBUF copy: "dve" | "act" | "split"
    skip=(),          # stages to skip (timing ablation only; wrong results)
)
_CFG = dict(CFG_DEFAULTS)


def build_module(repeat: int = 1, cfg: dict | None = None):
    global _CFG
    saved = _CFG
    _CFG = dict(CFG_DEFAULTS)
    if cfg:
        _CFG.update(cfg)
    try:
        return _build_module_inner(repeat)
    finally:
        _CFG = saved


def _build_module_inner(repeat: int):
    C = _CFG
    GROUP = C["group"]
    NGROUPS = NTILES // GROUP
    SKIP = set(C["skip"])

    nc = bacc.Bacc("TRN2", target_bir_lowering=False, debug=False)

    x_d = nc.dram_tensor("x", [TOK_PER_CORE, D], BF16, kind="ExternalInput")
    wt_d = nc.dram_tensor("wt", [D, O], F32, kind="ExternalInput")
    b_d = nc.dram_tensor("b", [O], F32, kind="ExternalInput")
    y_d = nc.dram_tensor("y", [TOK_PER_CORE, O], BF16, kind="ExternalOutput")

    x_r = x_d.ap().rearrange("(a p) d -> p a d", p=P)    # [128, 32, 1024]
    y_r = y_d.ap().rearrange("(a p) d -> p a d", p=P)
    wt_r = wt_d.ap().rearrange("(c p) o -> p c o", p=P)  # [128, 8, 1024]
    b_r = b_d.ap().rearrange("(o d) -> o d", o=1)        # [1, 1024]

    with tile.TileContext(nc) as tc:
        import contextlib

        with contextlib.ExitStack() as ctx:
            consts = ctx.enter_context(tc.tile_pool(name="consts", bufs=1))
            wpool = ctx.enter_context(tc.tile_pool(name="wpool", bufs=1))
            wtpool = ctx.enter_context(tc.tile_pool(name="wtpool", bufs=1))
            xpool = ctx.enter_context(tc.tile_pool(name="xpool", bufs=4))
            scr = ctx.enter_context(tc.tile_pool(name="scr", bufs=2))
            tpool = ctx.enter_context(tc.tile_pool(name="tpool", bufs=3))
            qpool = ctx.enter_context(tc.tile_pool(name="qpool", bufs=4))
            xtpool = ctx.enter_context(tc.tile_pool(name="xtpool", bufs=6))
            ypool = ctx.enter_context(tc.tile_pool(name="ypool", bufs=3))
            stats = ctx.enter_context(tc.tile_pool(name="stats", bufs=3))
            pspool = ctx.enter_context(
                tc.tile_pool(name="pspool", bufs=C["psbufs"], space="PSUM")
            )
            wps = ctx.enter_context(tc.tile_pool(name="wps", bufs=1, space="PSUM"))
            xps = None
            if C["transp"] == "pe":
                xps = ctx.enter_context(
                    tc.tile_pool(name="xps", bufs=2, space="PSUM")
                )

            # ---------------- constants ----------------
            ones_row = consts.tile([1, P], BF16)
            nc.vector.memset(ones_row, 1.0)
            ones128 = consts.tile([P, P], F32)
            nc.vector.memset(ones128, 1.0)
            if C["transp"] == "pe":
                from concourse.masks import make_identity

                identity_bf = consts.tile([P, P], FP16)
                make_identity(nc, identity_bf)

            # ---------------- weight/bias prep ----------------
            def emit_prep():
                # bias first; must not queue behind the 2MB w load
                b_sb = consts.tile([1, O], F32)
                nc.sync.dma_start(out=b_sb, in_=b_r)

                w_sb = wpool.tile([P, DCH, O], F32)
                for half in range(4):
                    nc.sync.dma_start(
                        out=w_sb[:, half * 2 : half * 2 + 2, :],
                        in_=wt_r[:, half * 2 : half * 2 + 2, :],
                    )

                # per-partition-row sums; split across ACT and DVE
                wsum = consts.tile([P, DCH], F32)
                wabs = consts.tile([P, DCH], F32)
                for r in range(DCH):
                    if r % 2 == 0:
                        dump = scr.tile([P, O], BF16, tag="sq")
                        nc.scalar.activation(
                            out=dump, in_=w_sb[:, r, :], func=ACTF.Copy,
                            accum_out=wsum[:, r : r + 1],
                        )
                    else:
                        nc.vector.tensor_reduce(
                            out=wsum[:, r : r + 1], in_=w_sb[:, r, :],
                            axis=mybir.AxisListType.X, op=ALU.add,
                        )
                for r in range(DCH):
                    nc.vector.tensor_reduce(
                        out=wabs[:, r : r + 1], in_=w_sb[:, r, :],
                        axis=mybir.AxisListType.X, op=ALU.add,
                        apply_absolute_value=True,
                    )
                w12 = consts.tile([P, 2], F32)
                nc.vector.tensor_reduce(
                    out=w12[:, 0:1], in_=wsum, axis=mybir.AxisListType.X,
                    op=ALU.add,
                )
                nc.vector.tensor_reduce(
                    out=w12[:, 1:2], in_=wabs, axis=mybir.AxisListType.X,
                    op=ALU.add,
                )
                # cross-partition reduce + broadcast in one f32 ones-matmul
                # (col 2 is reused later for the b_scale broadcast)
                statps = wps.tile([P, 4], F32, tag="stat", name="statps")
                nc.tensor.matmul(
                    statps[:, 0:2], lhsT=ones128, rhs=w12,
                    start=True, stop=True,
                )
                neg_mean_w = consts.tile([P, 1], F32)
                w_scale = consts.tile([P, 1], F32)
                nc.vector.tensor_scalar(
                    out=neg_mean_w, in0=statps[:, 0:1],
                    scalar1=-1.0 / float(O * D), scalar2=None, op0=ALU.mult,
                )
                nc.vector.tensor_scalar(
                    out=w_scale, in0=statps[:, 1:2],
                    scalar1=1.0 / float(O * D), scalar2=None, op0=ALU.mult,
                )

                # wqT[:, c, :] = Sign(wT_c - mean) straight from SBUF
                wqT = wtpool.tile([P, DCH, O], FP16)
                for c in range(DCH):
                    nc.scalar.activation(
                        out=wqT[:, c, :], in_=w_sb[:, c, :], func=ACTF.Sign,
                        bias=neg_mean_w, scale=1.0,
                    )

                # bias stats
                bsum = consts.tile([1, 1], F32)
                babs = consts.tile([1, 1], F32)
                nc.vector.tensor_reduce(
                    out=bsum, in_=b_sb, axis=mybir.AxisListType.X, op=ALU.add
                )
                nc.vector.tensor_reduce(
                    out=babs, in_=b_sb, axis=mybir.AxisListType.X, op=ALU.add,
                    apply_absolute_value=True,
                )
                neg_mean_b = consts.tile([1, 1], F32)
                b_scale1 = consts.tile([1, 1], F32)
                nc.vector.tensor_scalar(
                    out=neg_mean_b, in0=bsum, scalar1=-1.0 / float(O),
                    scalar2=None, op0=ALU.mult,
                )
                nc.vector.tensor_scalar(
                    out=b_scale1, in0=babs, scalar1=1.0 / float(O),
                    scalar2=None, op0=ALU.mult,
                )
                bq = consts.tile([1, O], BF16)
                nc.scalar.activation(
                    out=bq, in_=b_sb, func=ACTF.Sign, bias=neg_mean_b, scale=1.0
                )

                # PSUM seed row: b_q - 1536 * rowsum(w_q) per output column.
                # The quant step emits x_q + 1536 (fp16 magic), so the matmul
                # adds 1536*rowsum(wq) per column; the seed cancels it
                # exactly (all-integer f32).
                ones_colp = consts.tile([P, 1], FP16)
                nc.vector.memset(ones_colp, 1.0)
                rsh = wps.tile([1, 512], F32, tag="rs", name="rsps")
                seedrow = consts.tile([1, O], F32)
                for h in range(2):
                    for c in range(DCH):
                        nc.tensor.matmul(
                            rsh,
                            lhsT=ones_colp,
                            rhs=wqT[:, c, h * 512 : (h + 1) * 512],
                            start=(c == 0), stop=(c == DCH - 1),
                        )
                    nc.vector.tensor_scalar(
                        out=seedrow[:, h * 512 : (h + 1) * 512], in0=rsh,
                        scalar1=-OFFH, scalar2=None, op0=ALU.mult,
                    )
                bqf = consts.tile([1, O], F32)
                nc.vector.tensor_copy(out=bqf, in_=bq)
                seedrow2 = consts.tile([1, O], F32)
                nc.vector.tensor_tensor(
                    out=seedrow2, in0=seedrow, in1=bqf, op=ALU.add
                )
                # broadcast to all 128 partitions (f32 for exactness),
                # borrowing a main-loop PSUM buffer
                ones_col_f = consts.tile([1, P], F32)
                nc.vector.memset(ones_col_f, 1.0)
                bps = pspool.tile([P, O], F32, tag="ps", name="bps")
                for h in range(2):
                    nc.tensor.matmul(
                        bps[:, h * 512 : (h + 1) * 512],
                        lhsT=ones_col_f,
                        rhs=seedrow2[:, h * 512 : (h + 1) * 512],
                        start=True, stop=True,
                    )
                bqb = consts.tile([P, O], F32)
                nc.scalar.copy(out=bqb, in_=bps)

                # invc = 1 / (127 * w_scale * b_scale), broadcast to [128,1]
                nc.tensor.matmul(
                    statps[:, 2:3], lhsT=ones_col_f, rhs=b_scale1,
                    start=True, stop=True,
                )
                wb = consts.tile([P, 1], F32)
                nc.vector.tensor_tensor(
                    out=wb, in0=w_scale, in1=statps[:, 2:3], op=ALU.mult
                )
                wb127 = consts.tile([P, 1], F32)
                nc.vector.tensor_scalar(
                    out=wb127, in0=wb, scalar1=127.0, scalar2=None, op0=ALU.mult
                )
                invc = consts.tile([P, 1], F32)
                nc.vector.reciprocal(out=invc, in_=wb127)
                return wqT, bq, bqb, invc

            # ---------------- main loop ----------------
            # Software-pipelined: tile t's quant/transpose/PSUM-seed (front)
            # is emitted one tile AHEAD of tile t's matmuls+epilogue (back),
            # so the PE never waits for the xqT copy-back, and the ACT
            # stream orders seed(t+1) BEFORE epi(t).
            def emit_load(g):
                ldeng = nc.sync if C["load"] == "sp" else nc.gpsimd
                xg = xpool.tile([P, GROUP, D], BF16, tag="xg", name=f"xg_{g}")
                ldeng.dma_start(
                    out=xg, in_=x_r[:, g * GROUP : (g + 1) * GROUP, :]
                )
                sumsq = stats.tile([P, GROUP], F32, tag="sumsq", name=f"sumsq_{g}")
                amax = stats.tile([P, GROUP], F32, tag="amax", name=f"amax_{g}")
                return [xg, sumsq, amax, None, None]

            def emit_sqamax(st, j):
                xg, sumsq, amax = st[0], st[1], st[2]
                if "stats" in SKIP:
                    return
                sq = scr.tile([P, D], BF16, tag="sq")
                nc.scalar.activation(
                    out=sq, in_=xg[:, j, :], func=ACTF.Square,
                    accum_out=sumsq[:, j : j + 1],
                )
                nc.vector.tensor_reduce(
                    out=amax[:, j : j + 1], in_=xg[:, j, :],
                    axis=mybir.AxisListType.X, op=ALU.max,
                    apply_absolute_value=True,
                )

            def emit_chain(st, prep):
                wqT, bq, bqb, invc = prep
                xg, sumsq, amax = st[0], st[1], st[2]
                # per-token scalar chain on [128, GROUP]
                m = stats.tile([P, GROUP], F32, tag="m")
                gsc = stats.tile([P, GROUP], F32, tag="gsc")
                if "stats" in SKIP:
                    nc.vector.memset(m, 1.0)
                    nc.vector.memset(gsc, 1.0)
                    st[3], st[4] = m, gsc
                    return
                ssq = stats.tile([P, GROUP], F32)
                nc.vector.tensor_scalar(
                    out=ssq, in0=sumsq, scalar1=EPS_NORM_SQ, scalar2=None,
                    op0=ALU.max,
                )
                u = stats.tile([P, GROUP], F32)
                nc.vector.reciprocal(out=u, in_=ssq)
                v = stats.tile([P, GROUP], F32)
                nc.scalar.activation(out=v, in_=u, func=ACTF.Sqrt)
                for _ in range(2):  # Newton rsqrt refinement
                    rr = stats.tile([P, GROUP], F32, tag="rr")
                    nc.vector.tensor_tensor(out=rr, in0=v, in1=v, op=ALU.mult)
                    qq = stats.tile([P, GROUP], F32, tag="qq")
                    nc.vector.tensor_tensor(out=qq, in0=rr, in1=ssq, op=ALU.mult)
                    ww = stats.tile([P, GROUP], F32, tag="ww")
                    nc.vector.tensor_scalar(
                        out=ww, in0=qq, scalar1=-0.5, scalar2=1.5,
                        op0=ALU.mult, op1=ALU.add,
                    )
                    v2 = stats.tile([P, GROUP], F32, tag="vv")
                    nc.vector.tensor_tensor(out=v2, in0=v, in1=ww, op=ALU.mult)
                    v = v2

                am = stats.tile([P, GROUP], F32)
                nc.vector.tensor_scalar(
                    out=am, in0=amax, scalar1=1e-30, scalar2=None, op0=ALU.max,
                )
                im = stats.tile([P, GROUP], F32)
                nc.vector.reciprocal(out=im, in_=am)
                nc.vector.tensor_scalar(
                    out=m, in0=im, scalar1=127.0, scalar2=None, op0=ALU.mult,
                )
                ax1 = stats.tile([P, GROUP], F32)
                nc.vector.tensor_tensor(out=ax1, in0=amax, in1=v, op=ALU.mult)
                axnc = stats.tile([P, GROUP], F32)
                nc.vector.tensor_scalar(
                    out=axnc, in0=ax1, scalar1=DIM_SCALE, scalar2=EPS_SCALE,
                    op0=ALU.mult, op1=ALU.max,
                )
                nc.vector.tensor_scalar(
                    out=gsc, in0=axnc, scalar1=invc, scalar2=None, op0=ALU.mult,
                )
                st[3], st[4] = m, gsc

            def emit_front(t, st, prep):
                wqT, bq, bqb, invc = prep
                teng = nc.scalar if C["tring"] == "act" else nc.sync
                xg, m, gsc = st[0], st[3], st[4]
                j = t % GROUP

                # quantize: xq_pre = x_q + 1536, fp16 (ulp=1 in [1024,2048),
                # so the fp16 output cast IS the round-half-even)
                xq = qpool.tile([P, D], FP16)
                if "quant" in SKIP:
                    nc.vector.memset(xq, OFFH)
                elif C["qmode"] == "dve1":
                    nc.vector.tensor_scalar(
                        out=xq, in0=xg[:, j, :], scalar1=m[:, j : j + 1],
                        scalar2=OFFH, op0=ALU.mult, op1=ALU.add,
                    )
                else:
                    # f32 magic round on DVE, then exact fp16 cast (+1536)
                    t1 = tpool.tile([P, D], F32)
                    nc.vector.tensor_scalar(
                        out=t1, in0=xg[:, j, :], scalar1=m[:, j : j + 1],
                        scalar2=MAGIC, op0=ALU.mult, op1=ALU.add,
                    )
                    if C["qmode"] == "actcast":
                        nc.scalar.activation(
                            out=xq, in_=t1, func=ACTF.Copy,
                            bias=OFFH - MAGIC, scale=1.0,
                        )
                    else:
                        nc.vector.tensor_scalar(
                            out=xq, in0=t1, scalar1=MAGIC - OFFH,
                            scalar2=None, op0=ALU.subtract,
                        )

                # transpose -> xqT[p, c, t] = xq[t, c*128+p]
                xqT = xtpool.tile([P, DCH, P], FP16)
                if "transpose" in SKIP:
                    nc.vector.memset(xqT, OFFH)
                elif C["transp"] == "xbar":
                    teng.dma_start_transpose(xqT, xq)
                else:
                    ptx = xps.tile([P, D], FP16, tag="xtp")
                    for c in range(DCH):
                        nc.tensor.transpose(
                            ptx[:, c * P : (c + 1) * P],
                            xq[:, c * P : (c + 1) * P],
                            identity_bf,
                        )
                    xqT_flat = xqT.rearrange("p c t -> p (c t)")
                    if C["cp"] == "dve":
                        nc.vector.tensor_copy(out=xqT_flat, in_=ptx)
                    elif C["cp"] == "act":
                        nc.scalar.copy(out=xqT_flat, in_=ptx)
                    else:
                        nc.vector.tensor_copy(
                            out=xqT_flat[:, 0:512], in_=ptx[:, 0:512]
                        )
                        nc.scalar.copy(
                            out=xqT_flat[:, 512:1024], in_=ptx[:, 512:1024]
                        )

                # seed PSUM with bias - 384*rowsum(wq) (ACT)
                ps = pspool.tile([P, O], F32, tag="ps")
                if "mm" not in SKIP:
                    nc.scalar.copy(out=ps, in_=bqb)
                return xqT, ps, gsc, j

            def emit_back(t, rec, prep, ycarry):
                wqT, bq, bqb, invc = prep
                steng = nc.sync if C["store"] == "sp" else nc.scalar
                xqT, ps, gsc, j = rec
                pss = [ps[:, 0:512], ps[:, 512:1024]]

                if "mm" not in SKIP:
                    for c in range(DCH):
                        for h in range(2):
                            nc.tensor.matmul(
                                pss[h],
                                lhsT=xqT[:, c, :],
                                rhs=wqT[:, c, h * 512 : (h + 1) * 512],
                                start=False,
                                stop=(c == DCH - 1),
                                skip_group_check=(c == 0),
                            )

                # dequant + store (bf16 out), batched x2
                if t % 2 == 0:
                    ycarry["yt2"] = ypool.tile(
                        [P, 2, O], BF16, tag="yt", name=f"yt2_{t}"
                    )
                yt2 = ycarry["yt2"]
                if "epi" not in SKIP and "mm" not in SKIP:
                    nc.scalar.activation(
                        out=yt2[:, t % 2, :], in_=ps, func=ACTF.Copy,
                        bias=0.0, scale=gsc[:, j : j + 1],
                    )
                else:
                    nc.vector.memset(yt2[:, t % 2, :], 0.0)
                if t % 2 == 1:
                    steng.dma_start(
                        out=y_r[:, t - 1 : t + 1, :], in_=yt2
                    )

            def main_loop(prep):
                ycarry = {}
                pending = None
                states = {}
                # prologue: group 0 stats upfront; loads go 2 groups ahead
                states[0] = emit_load(0)
                if NGROUPS > 1:
                    states[1] = emit_load(1)
                for j in range(GROUP):
                    emit_sqamax(states[0], j)
                emit_chain(states[0], prep)
                for t in range(NTILES):
                    g, k = divmod(t, GROUP)
                    if k == 0 and g + 2 < NGROUPS:
                        states[g + 2] = emit_load(g + 2)
                    rec = emit_front(t, states[g], prep)
                    # spread next group's stats one op per tile slot
                    if g + 1 < NGROUPS:
                        emit_sqamax(states[g + 1], k)
                        if k == GROUP - 1:
                            emit_chain(states[g + 1], prep)
                    if pending is not None:
                        emit_back(t - 1, pending, prep, ycarry)
                    pending = rec
                    states.pop(g - 1, None)
                emit_back(NTILES - 1, pending, prep, ycarry)

            if repeat == 1:
                prep = emit_prep()
                main_loop(prep)
            else:
                prep = emit_prep()
                with tc.For_i(0, repeat, 1):
                    main_loop(prep)

    nc.compile()
    return nc


_NC_CACHE = None


def _get_module():
    global _NC_CACHE
    if _NC_CACHE is None:
        _NC_CACHE = build_module()
    return _NC_CACHE


def make_in_map(x_core: np.ndarray, w: np.ndarray, b: np.ndarray) -> dict:
    """Per-core input map: x block in bf16, w transposed+bf16, b f32."""
    return {
        "x": np.ascontiguousarray(x_core, dtype=BF16NP),
        "wt": np.ascontiguousarray(np.asarray(w, dtype=np.float32).T),
        "b": np.ascontiguousarray(b, dtype=np.float32),
    }


def kernel(x: np.ndarray, w: np.ndarray, b: np.ndarray) -> np.ndarray:
    assert x.shape == (B, S, D) and w.shape == (O, D) and b.shape == (O,)
    nc = _get_module()

    xf = np.asarray(x, dtype=np.float32).reshape(TOKENS, D).astype(BF16NP)
    wt = np.ascontiguousarray(np.asarray(w, dtype=np.float32).T)
    bf = np.ascontiguousarray(b, dtype=np.float32)

    in_maps = [
        {
            "x": np.ascontiguousarray(
                xf[i * TOK_PER_CORE : (i + 1) * TOK_PER_CORE]
            ),
            "wt": wt,
            "b": bf,
        }
        for i in range(N_CORES)
    ]
    res = run_bass_kernel_spmd(nc, in_maps, core_ids=list(range(N_CORES)))
    out = np.concatenate([res.results[i]["y"] for i in range(N_CORES)], axis=0)
    return out.reshape(B, S, O).astype(np.float32)
